# revision 20
# baseline (speedup 1.0000x reference)
"""Trainium2 Bass kernel for a GQA attention layer (B=2, S=2048, D=4096,
32 Q heads / 8 KV heads, rotary, additive causal mask), SPMD across 8
NeuronCores.

Sharding: core c = (batch b=c//4, stripe j=c%4) owns the STRIDED query
chunk set {j, 4+j, 8+j, 12+j} (128 tokens each, position-major order).
This balances causal work exactly across cores while keeping one uniform
SPMD program: at local query position p the schedule always covers key
chunks 0..4p+3; chunks above the core's own diagonal arrive fully masked
in that core's mask data and contribute exp(-inf)=0.

K/V projections are computed for local tokens only and shared within
each batch's 4 cores via one AllGather (global key chunk i lives in
gathered slot r=i%4, sub-chunk i//4). Attention computes transposed
scores (S^T = K^T-chunk.T @ Q^T); for key chunk i only the query suffix
from position i//4 is computed (one matmul of width 512-128*(i//4)), and
only the first 128-column block (the diagonal) gets a mask add on the
DVE. exp(S^T) feeds the A*V matmul as the moving operand with
region-aligned per-position accumulation; the softmax denominator
accumulates on the PE via an all-ones stationary operand and is applied
after A*V (logits are bounded, so no max subtraction). The wo projection
produces each core's 512 output rows, unshuffled on the host.

Weights are host-packed so every [128, w] stationary tile is a single
contiguous DMA.
"""

import os
import sys
from contextlib import ExitStack
from dataclasses import dataclass

import numpy as np

if os.path.isdir("/opt/trn_rl_repo") and "/opt/trn_rl_repo" not in sys.path:
    sys.path.insert(0, "/opt/trn_rl_repo")

import ml_dtypes

import concourse.bass as bass
import concourse.mybir as mybir
import concourse.tile as tile
from concourse import bacc
from concourse.bass_utils import run_bass_kernel_spmd

BF16 = mybir.dt.bfloat16
F32 = mybir.dt.float32
NPBF16 = ml_dtypes.bfloat16
P = 128


@dataclass(frozen=True)
class Cfg:
    S: int = 2048      # full sequence
    D: int = 4096      # model dim
    NH: int = 32       # query heads
    NKV: int = 8       # kv heads
    HD: int = 128      # head dim (must equal P)

    @property
    def T(self):
        return self.S // 4

    @property
    def TS(self):
        return self.T // P

    @property
    def DT(self):
        return self.D // P

    @property
    def NREP(self):
        return self.NH // self.NKV


FULL = Cfg()


def groups_of3(n):
    return [list(range(k, min(k + 3, n))) for k in range(0, n, 3)]


def pack_colgroups(wT, groups, DT):
    """wT: [D, E] contraction-major. Flat layout: [group][d][128, w_g]
    contiguous blocks."""
    blocks = []
    for grp in groups:
        c0, w = grp[0] * P, len(grp) * P
        for d in range(DT):
            blocks.append(
                np.ascontiguousarray(wT[d * P:(d + 1) * P, c0:c0 + w]).reshape(-1))
    return np.concatenate(blocks)


def build_nc(cfg: Cfg):
    S, D, NH, NKV, HD = cfg.S, cfg.D, cfg.NH, cfg.NKV, cfg.HD
    T, TS, DT = cfg.T, cfg.TS, cfg.DT
    KVW = NKV * HD
    NCH = 4 * TS
    NEH = KVW // 512               # V feature halves
    NDO = D // 512                 # wo output column groups
    SCALE = float(np.float32(1.0) / np.float32(np.sqrt(np.float32(HD))))

    kgroups = groups_of3(NKV)
    qgroups = groups_of3(NH)
    vgroups = [(eh, tss) for eh in range(NEH) for tss in groups_of3(TS)]

    nc = bacc.Bacc("TRN2", target_bir_lowering=False, debug=False, num_devices=8)

    xt_d = nc.dram_tensor("xt", [D, T], BF16, kind="ExternalInput")
    wqp_d = nc.dram_tensor("wqp", [D * NH * HD], BF16, kind="ExternalInput")
    wkp_d = nc.dram_tensor("wkp", [D * KVW], BF16, kind="ExternalInput")
    wvp_d = nc.dram_tensor("wvp", [D * KVW], BF16, kind="ExternalInput")
    wop_d = nc.dram_tensor("wop", [NH * HD * D], BF16, kind="ExternalInput")
    cost_d = nc.dram_tensor("cost", [HD, T], F32, kind="ExternalInput")
    sint_d = nc.dram_tensor("sint", [HD, T], F32, kind="ExternalInput")
    maskt_d = nc.dram_tensor("maskt", [S, T], BF16, kind="ExternalInput")
    swap_d = nc.dram_tensor("swapm", [P, P], BF16, kind="ExternalInput")
    onesmat_d = nc.dram_tensor("onesmat", [P, P], BF16, kind="ExternalInput")
    out_d = nc.dram_tensor("out", [T, D], F32, kind="ExternalOutput")

    def grp_offsets(groups):
        offs = []
        off = 0
        for grp in groups:
            offs.append(off)
            off += DT * P * len(grp) * P
        return offs

    qoffs = grp_offsets(qgroups)
    koffs = grp_offsets(kgroups)

    wqp = wqp_d.ap()
    wkp = wkp_d.ap()
    wvp = wvp_d.ap()
    wop = wop_d.ap()

    with tile.TileContext(nc) as tc, ExitStack() as ctx:
        persist = ctx.enter_context(tc.tile_pool(name="persist", bufs=1))
        wpool = ctx.enter_context(tc.tile_pool(name="wpool", bufs=3))
        dramp = ctx.enter_context(tc.tile_pool(name="dramp", bufs=1, space="DRAM"))

        # ---- constants ----
        swap_sb = persist.tile([P, P], BF16, name="swap_sb")
        nc.sync.dma_start(swap_sb[:], swap_d.ap()[:])
        cost_sb = persist.tile([HD, T], F32, name="cost_sb")
        nc.sync.dma_start(cost_sb[:], cost_d.ap()[:])
        sint_sb = persist.tile([HD, T], F32, name="sint_sb")
        nc.sync.dma_start(sint_sb[:], sint_d.ap()[:])
        onesmat_sb = persist.tile([P, P], BF16, name="onesmat_sb")
        nc.sync.dma_start(onesmat_sb[:], onesmat_d.ap()[:])

        kvink = dramp.tile([KVW, T], BF16, name="kvink")
        kvoutk = dramp.tile([4 * KVW, T], BF16, name="kvoutk")
        kvinv = dramp.tile([KVW, T], BF16, name="kvinv")
        kvoutv = dramp.tile([4 * KVW, T], BF16, name="kvoutv")
        kvinv_flat = kvinv[:].rearrange("a b -> (a b)")
        kvoutv_flat = kvoutv[:].rearrange("a b -> (a b)")

        qt = [persist.tile([P, T], BF16, name=f"qt_{h}") for h in range(NH)]

        with tc.tile_pool(name="xtp", bufs=1) as xtp, \
             tc.tile_pool(name="rot", bufs=2) as rot, \
             tc.tile_pool(name="wproj", bufs=1) as wproj, \
             tc.tile_pool(name="psP", bufs=1, space="PSUM") as psP:

            def rotary(raw_ps, dst_bf16, nm):
                """Interleaved rotary on a [P, T] feature-transposed PSUM tile."""
                raw = rot.tile([P, T], BF16, tag="raw", bufs=6, name=f"raw_{nm}")
                nc.scalar.copy(raw[:], raw_ps[:])
                sw_ps = psP.tile([P, T], F32, tag="swp", bufs=2, name=f"swp_{nm}")
                nc.tensor.matmul(sw_ps[:], swap_sb[:], raw[:], start=True, stop=True)
                t1 = rot.tile([P, T], F32, tag="t1", bufs=4, name=f"t1_{nm}")
                nc.vector.tensor_mul(t1[:], raw[:], cost_sb[:])
                t2 = rot.tile([P, T], F32, tag="t2", bufs=4, name=f"t2_{nm}")
                nc.vector.tensor_mul(t2[:], sw_ps[:], sint_sb[:])
                nc.vector.tensor_add(dst_bf16[:], t1[:], t2[:])

            xt_sb = [xtp.tile([P, T], BF16, name=f"xt_sb_{d}") for d in range(DT)]
            xt_loaded = [False] * DT

            def load_xt(d):
                if not xt_loaded[d]:
                    (nc.sync if d % 2 == 0 else nc.scalar).dma_start(
                        xt_sb[d][:], xt_d.ap()[d * P:(d + 1) * P, :])
                    xt_loaded[d] = True

            # ---- K^T projection (local tokens) + rotary ----
            ktloc = [xtp.tile([P, T], BF16, name=f"ktloc_{kvh}")
                     for kvh in range(NKV)]
            for gi, grp in enumerate(kgroups):
                w = len(grp) * P
                kps = [psP.tile([P, T], F32, tag=f"pj{j}", bufs=2,
                                name=f"kps_{gi}_{j}") for j in range(len(grp))]
                for d in range(DT):
                    wrow = wproj.tile([P, 3 * P], BF16, tag="wkv", bufs=12,
                                      name=f"wk_{gi}_{d}")
                    off = koffs[gi] + d * P * w
                    (nc.sync if d % 2 == 0 else nc.scalar).dma_start(
                        wrow[:, :w],
                        wkp[off:off + P * w].rearrange("(p f) -> p f", p=P))
                    load_xt(d)
                    for j in range(len(grp)):
                        nc.tensor.matmul(
                            kps[j][:], wrow[:, j * HD:(j + 1) * HD], xt_sb[d][:],
                            start=(d == 0), stop=(d == DT - 1))
                for j, kvh in enumerate(grp):
                    rotary(kps[j], ktloc[kvh], f"k{kvh}")

            # ---- K^T pack + AllGather (overlaps the V projection) ----
            for kvh in range(NKV):
                nc.sync.dma_start(kvink[kvh * HD:(kvh + 1) * HD, :], ktloc[kvh][:])
            nc.gpsimd.collective_compute(
                "AllGather",
                mybir.AluOpType.bypass,
                replica_groups=[[0, 1, 2, 3], [4, 5, 6, 7]],
                ins=[kvink[:].opt()],
                outs=[kvoutk[:].opt()],
            )

            # ---- V projection (local tokens), [token, feature] layout ----
            vtloc = [xtp.tile([P, KVW], BF16, name=f"vtloc_{ts}")
                     for ts in range(TS)]
            for gi, (eh, tss) in enumerate(vgroups):
                vps = [psP.tile([P, 512], F32, tag=f"pj{j}", bufs=2,
                                name=f"vps_{gi}_{j}") for j in range(len(tss))]
                for d in range(DT):
                    wrow = wproj.tile([P, 512], BF16, tag="wvr", bufs=12,
                                      name=f"wv_{gi}_{d}")
                    off = (eh * DT + d) * P * 512
                    (nc.sync if d % 2 == 0 else nc.scalar).dma_start(
                        wrow[:],
                        wvp[off:off + P * 512].rearrange("(p f) -> p f", p=P))
                    for j, ts in enumerate(tss):
                        nc.tensor.matmul(
                            vps[j][:], xt_sb[d][:, ts * P:(ts + 1) * P], wrow[:],
                            start=(d == 0), stop=(d == DT - 1))
                for j, ts in enumerate(tss):
                    nc.scalar.copy(vtloc[ts][:, eh * 512:(eh + 1) * 512], vps[j][:])

            # ---- V pack + AllGather (overlaps the Q projection) ----
            # V is stored as [kvh][ts] blocks of [128 tokens, 128 features]
            # so the gathered per-(kvh,chunk) slices are contiguous.
            for kvh in range(NKV):
                for ts in range(TS):
                    off = (kvh * TS + ts) * P * HD
                    nc.sync.dma_start(
                        kvinv_flat[off:off + P * HD]
                        .rearrange("(p f) -> p f", p=P),
                        vtloc[ts][:, kvh * HD:(kvh + 1) * HD])

            nc.gpsimd.collective_compute(
                "AllGather",
                mybir.AluOpType.bypass,
                replica_groups=[[0, 1, 2, 3], [4, 5, 6, 7]],
                ins=[kvinv[:].opt()],
                outs=[kvoutv[:].opt()],
            )

            # ---- Q^T projection + rotary (overlaps the AllGather) ----
            for gi, grp in enumerate(qgroups):
                w = len(grp) * P
                qps = [psP.tile([P, T], F32, tag=f"pj{j}", bufs=2,
                                name=f"qps_{gi}_{j}") for j in range(len(grp))]
                for d in range(DT):
                    wrow = wproj.tile([P, 3 * P], BF16, tag="wq", bufs=24,
                                      name=f"wq_{gi}_{d}")
                    off = qoffs[gi] + d * P * w
                    (nc.sync if d % 2 == 0 else nc.scalar).dma_start(
                        wrow[:, :w],
                        wqp[off:off + P * w].rearrange("(p f) -> p f", p=P))
                    for j in range(len(grp)):
                        nc.tensor.matmul(
                            qps[j][:], wrow[:, j * HD:(j + 1) * HD], xt_sb[d][:],
                            start=(d == 0), stop=(d == DT - 1))
                for j, h in enumerate(grp):
                    rotary(qps[j], qt[h], f"q{h}")

        tc.no_sync_barrier()

        # Diagonal-schedule mask blocks: key chunk i is masked (per-core
        # data) only against local query position i//4.
        maskt_sb = []
        for i in range(NCH):
            t = persist.tile([P, P], BF16, name=f"maskt_sb_{i}")
            p0 = i // 4
            nc.sync.dma_start(
                t[:], maskt_d.ap()[i * P:(i + 1) * P, p0 * P:(p0 + 1) * P])
            maskt_sb.append(t)

        # ---- attention, streaming gathered K^T / V per kv head ----
        # Key chunk i covers local query positions i//4 .. 3 (a contiguous
        # suffix of the position-major qt tile): one score matmul of width
        # T - 128*(i//4), mask add on its first 128-col block only.
        att = [persist.tile([P, T], BF16, name=f"att_{h}") for h in range(NH)]
        with tc.tile_pool(name="kvp", bufs=1) as kvp, \
             tc.tile_pool(name="atw", bufs=1) as work, \
             tc.tile_pool(name="psA", bufs=1, space="PSUM") as psA:

            def load_kv(kvh):
                ktl = {}
                vtl = {}
                for r in range(4):
                    kt_t = kvp.tile([P, T], BF16, tag="kt", bufs=8,
                                    name=f"kt_{kvh}_{r}")
                    nc.sync.dma_start(
                        kt_t[:],
                        kvoutk[r * KVW + kvh * HD: r * KVW + (kvh + 1) * HD, :])
                    ktl[r] = kt_t
                    for ts in range(TS):
                        i = 4 * ts + r   # global chunk owned by core r, pos ts
                        vt_t = kvp.tile([P, HD], BF16, tag="vts", bufs=2 * NCH,
                                        name=f"vt_{kvh}_{r}_{ts}")
                        off = r * KVW * T + (kvh * TS + ts) * P * HD
                        nc.sync.dma_start(
                            vt_t[:],
                            kvoutv_flat[off:off + P * HD]
                            .rearrange("(p f) -> p f", p=P))
                        vtl[i] = vt_t
                return ktl, vtl

            def emit_scores(hs, ktl):
                """Scores + strided exp + GPSIMD diagonal triangle multiply.
                Strips of a chunk pair sit at fixed T-column slots of a
                2-bank PSUM tile (a matmul may not cross a bank boundary)."""
                et = {0: {}, 1: {}}   # et[j][g, half] -> exp tile
                for g in range(4):
                    w = T - g * P
                    for half in range(2):
                        for j, h in enumerate(hs):
                            s2 = psA.tile([P, 2 * T], F32, tag="s", bufs=2,
                                          name=f"s_{h}_{g}_{half}")
                            for m in range(2):
                                i = 4 * g + 2 * half + m
                                nc.tensor.matmul(
                                    s2[:, m * T:m * T + w],
                                    ktl[i % 4][:, (i // 4) * P:(i // 4 + 1) * P],
                                    qt[h][:, g * P:],
                                    start=True, stop=True)
                            e2 = work.tile([P, 2 * T], BF16, tag="et",
                                           bufs=32, name=f"e_{h}_{g}_{half}")
                            nc.scalar.activation(
                                e2[:].rearrange("p (m c) -> p m c",
                                                m=2)[:, :, :w],
                                s2[:].rearrange("p (m c) -> p m c",
                                                m=2)[:, :, :w],
                                mybir.ActivationFunctionType.Exp,
                                scale=SCALE)
                            # causal cut: multiply the diagonal 128-col
                            # block by a {0,1} triangle (exact in bf16);
                            # runs on the otherwise-idle GPSIMD engine.
                            for m in range(2):
                                i = 4 * g + 2 * half + m
                                eng = nc.gpsimd if i % 2 == 0 else nc.vector
                                eng.tensor_mul(
                                    e2[:, m * T:m * T + P],
                                    e2[:, m * T:m * T + P],
                                    maskt_sb[i][:])
                            et[j][(g, half)] = e2
                return et

            def emit_av(hs, av_ps, zb_ps, et, vtl):
                """A*V and Z accumulation + normalization for one head pair."""
                for j, h in enumerate(hs):
                    for stat_ones in (False, True):
                        dst = zb_ps[j] if stat_ones else av_ps[j]
                        for i in range(NCH):
                            g, rem = i // 4, i % 4
                            e2 = et[j][(g, rem // 2)]
                            m = rem % 2
                            stat = onesmat_sb[:] if stat_ones else vtl[i][:]
                            # One suffix-wide matmul per chunk: it only
                            # touches positions >= g, so per-position
                            # accumulation falls out of the width. start
                            # fires once (chunk 0 spans the full bank),
                            # stop once on the final chunk.
                            nc.tensor.matmul(
                                dst[:, g * P:],
                                stat,
                                e2[:, m * T:m * T + (T - g * P)],
                                start=(i == 0), stop=(i == NCH - 1))
                    rzb = work.tile([P, T], F32, tag="rzbs", bufs=2,
                                    name=f"rzbs_{h}")
                    nc.vector.reciprocal_approx_fast(out=rzb[:],
                                                     in_=zb_ps[j][:])
                    nc.vector.tensor_mul(att[h][:], av_ps[j][:], rzb[:])

            # Software pipeline across (kvh, sub): AV/Z of the previous head
            # pair is emitted after the scores of the current one, so the ACT
            # exp of pair k overlaps the PE AV/Z of pair k-1 instead of
            # stalling the PE (which also kept re-tripping the HAM throttle).
            pending = None
            for kvh in range(NKV):
                ktl, vtl = load_kv(kvh)
                for sub in range(cfg.NREP // 2):
                    hs = [kvh * cfg.NREP + sub * 2, kvh * cfg.NREP + sub * 2 + 1]
                    av_ps = {}
                    zb_ps = {}
                    for j, h in enumerate(hs):
                        av_ps[j] = psA.tile([P, T], F32, tag="av", bufs=2,
                                            name=f"av_{h}")
                        zb_ps[j] = psA.tile([P, T], F32, tag="zb", bufs=2,
                                            name=f"zb_{h}")
                    et = emit_scores(hs, ktl)
                    if pending is not None:
                        emit_av(*pending)
                    pending = (hs, av_ps, zb_ps, et, vtl)
            emit_av(*pending)

        tc.no_sync_barrier()

        # ---- output projection ----
        with tc.tile_pool(name="osbp", bufs=1) as osbp, \
             tc.tile_pool(name="psW", bufs=1, space="PSUM") as psW:
            for douth in range(NDO):
                ops = [psW.tile([P, 512], F32, tag=f"pw{tt}", bufs=2,
                                name=f"ops_{douth}_{tt}") for tt in range(TS)]
                for e in range(NH):
                    wrow = wpool.tile([P, 512], BF16, tag="wo", bufs=12,
                                      name=f"wo_{douth}_{e}")
                    off = (douth * NH + e) * P * 512
                    (nc.sync, nc.scalar, nc.gpsimd)[e % 3].dma_start(
                        wrow[:],
                        wop[off:off + P * 512].rearrange("(p f) -> p f", p=P))
                    for tt in range(TS):
                        nc.tensor.matmul(
                            ops[tt][:], att[e][:, tt * P:(tt + 1) * P], wrow[:],
                            start=(e == 0), stop=(e == NH - 1))
                for tt in range(TS):
                    osb = osbp.tile([P, 512], F32, tag="osb", bufs=4,
                                    name=f"osb_{douth}_{tt}")
                    nc.scalar.copy(osb[:], ops[tt][:])
                    nc.sync.dma_start(
                        out_d.ap()[tt * P:(tt + 1) * P, douth * 512:(douth + 1) * 512],
                        osb[:])

    nc.compile()
    return nc


def owned_tokens(j, cfg: Cfg):
    """Strided query chunks {j, 4+j, 8+j, 12+j}, position-major."""
    return np.concatenate([
        np.arange(P) + P * (4 * p + j) for p in range(cfg.TS)])


def make_in_maps(x, freqs_cis, mask, wq, wk, wv, wo, cfg: Cfg):
    S, D, T, HD, DT = cfg.S, cfg.D, cfg.T, cfg.HD, cfg.DT
    NEH = cfg.NKV * HD // 512
    NDO = D // 512
    SCALE = np.float32(1.0) / np.float32(np.sqrt(np.float32(HD)))
    x = np.asarray(x, np.float32)
    fc = np.asarray(freqs_cis, np.float32)
    mask = np.asarray(mask, np.float32)
    wqt = np.asarray(wq, np.float32).T.astype(NPBF16)   # [D, NH*HD]
    wkt = np.asarray(wk, np.float32).T.astype(NPBF16)   # [D, KVW]
    wvt = np.asarray(wv, np.float32).T.astype(NPBF16)
    wot = np.asarray(wo, np.float32).T.astype(NPBF16)   # [NH*HD, D]

    wqp = pack_colgroups(wqt, groups_of3(cfg.NH), DT)
    wkp = pack_colgroups(wkt, groups_of3(cfg.NKV), DT)
    wvp = np.concatenate([
        np.ascontiguousarray(wvt[d * P:(d + 1) * P, eh * 512:(eh + 1) * 512])
        .reshape(-1)
        for eh in range(NEH) for d in range(DT)])
    wop = np.concatenate([
        np.ascontiguousarray(wot[e * P:(e + 1) * P, douth * 512:(douth + 1) * 512])
        .reshape(-1)
        for douth in range(NDO) for e in range(cfg.NH)])

    swapm = np.zeros((P, P), np.float32)
    for i in range(P // 2):
        swapm[2 * i, 2 * i + 1] = 1.0
        swapm[2 * i + 1, 2 * i] = 1.0
    swapm = swapm.astype(NPBF16)
    onesmat = np.ones((P, P), NPBF16)

    in_maps = []
    for c in range(8):
        b, j = c // 4, c % 4
        sl = owned_tokens(j, cfg)
        xt = np.ascontiguousarray(x[b, sl, :].T).astype(NPBF16)
        cost = np.repeat(fc[sl, :, 0].T, 2, axis=0).astype(np.float32)
        sint = np.repeat(fc[sl, :, 1].T, 2, axis=0).astype(np.float32)
        sint[0::2, :] *= -1.0
        # {0,1} visibility triangle (multiplied into exp(scores), exact in
        # bf16); only the 16 diagonal-schedule blocks are read on device
        maskt = np.ascontiguousarray((mask[sl, :] == 0.0).T.astype(np.float32)
                                     ).astype(NPBF16)
        in_maps.append({
            "xt": xt, "wqp": wqp, "wkp": wkp, "wvp": wvp, "wop": wop,
            "cost": np.ascontiguousarray(cost),
            "sint": np.ascontiguousarray(sint),
            "maskt": maskt, "swapm": swapm, "onesmat": onesmat,
        })
    return in_maps


_NC_CACHE = {}


def kernel_run(x, start_pos, freqs_cis, mask, wq, wk, wv, wo,
               cfg: Cfg = FULL, trace=False):
    in_maps = make_in_maps(x, freqs_cis, mask, wq, wk, wv, wo, cfg)
    if cfg not in _NC_CACHE:
        _NC_CACHE[cfg] = build_nc(cfg)
    nc = _NC_CACHE[cfg]
    res = run_bass_kernel_spmd(nc, in_maps, core_ids=list(range(8)), trace=trace)
    full = np.empty((2, cfg.S, cfg.D), np.float32)
    for c in range(8):
        b, j = c // 4, c % 4
        full[b, owned_tokens(j, cfg), :] = res.results[c]["out"]
    return full, res


def kernel(x, start_pos=None, freqs_cis=None, mask=None, wq=None, wk=None,
           wv=None, wo=None):
    full, _ = kernel_run(x, start_pos, freqs_cis, mask, wq, wk, wv, wo)
    return full



# revision 22
# speedup vs baseline: 1.0101x; 1.0101x over previous
"""Trainium2 Bass kernel for a GQA attention layer (B=2, S=2048, D=4096,
32 Q heads / 8 KV heads, rotary, additive causal mask), SPMD across 8
NeuronCores.

Sharding: core c = (batch b=c//4, stripe j=c%4) owns the STRIDED query
chunk set {j, 4+j, 8+j, 12+j} (128 tokens each, position-major order).
This balances causal work exactly across cores while keeping one uniform
SPMD program: at local query position p the schedule always covers key
chunks 0..4p+3; chunks above the core's own diagonal arrive fully masked
in that core's mask data and contribute exp(-inf)=0.

K/V projections are computed for local tokens only and shared within
each batch's 4 cores via one AllGather (global key chunk i lives in
gathered slot r=i%4, sub-chunk i//4). Attention computes transposed
scores (S^T = K^T-chunk.T @ Q^T); for key chunk i only the query suffix
from position i//4 is computed (one matmul of width 512-128*(i//4)), and
only the first 128-column block (the diagonal) gets a mask add on the
DVE. exp(S^T) feeds the A*V matmul as the moving operand with
region-aligned per-position accumulation; the softmax denominator
accumulates on the PE via an all-ones stationary operand and is applied
after A*V (logits are bounded, so no max subtraction). The wo projection
produces each core's 512 output rows, unshuffled on the host.

Weights are host-packed so every [128, w] stationary tile is a single
contiguous DMA.
"""

import os
import sys
from contextlib import ExitStack
from dataclasses import dataclass

import numpy as np

if os.path.isdir("/opt/trn_rl_repo") and "/opt/trn_rl_repo" not in sys.path:
    sys.path.insert(0, "/opt/trn_rl_repo")

import ml_dtypes

import concourse.bass as bass
import concourse.mybir as mybir
import concourse.tile as tile
from concourse import bacc
from concourse.bass_utils import run_bass_kernel_spmd

BF16 = mybir.dt.bfloat16
F32 = mybir.dt.float32
NPBF16 = ml_dtypes.bfloat16
P = 128


@dataclass(frozen=True)
class Cfg:
    S: int = 2048      # full sequence
    D: int = 4096      # model dim
    NH: int = 32       # query heads
    NKV: int = 8       # kv heads
    HD: int = 128      # head dim (must equal P)

    @property
    def T(self):
        return self.S // 4

    @property
    def TS(self):
        return self.T // P

    @property
    def DT(self):
        return self.D // P

    @property
    def NREP(self):
        return self.NH // self.NKV


FULL = Cfg()


def groups_of3(n):
    return [list(range(k, min(k + 3, n))) for k in range(0, n, 3)]


def pack_colgroups(wT, groups, DT):
    """wT: [D, E] contraction-major. Flat layout: [group][d][128, w_g]
    contiguous blocks."""
    blocks = []
    for grp in groups:
        c0, w = grp[0] * P, len(grp) * P
        for d in range(DT):
            blocks.append(
                np.ascontiguousarray(wT[d * P:(d + 1) * P, c0:c0 + w]).reshape(-1))
    return np.concatenate(blocks)


def build_nc(cfg: Cfg):
    S, D, NH, NKV, HD = cfg.S, cfg.D, cfg.NH, cfg.NKV, cfg.HD
    T, TS, DT = cfg.T, cfg.TS, cfg.DT
    KVW = NKV * HD
    NCH = 4 * TS
    NEH = KVW // 512               # V feature halves
    NDO = D // 512                 # wo output column groups
    SCALE = float(np.float32(1.0) / np.float32(np.sqrt(np.float32(HD))))

    kgroups = groups_of3(NKV)
    qgroups = groups_of3(NH)
    vgroups = [(eh, tss) for eh in range(NEH) for tss in groups_of3(TS)]

    nc = bacc.Bacc("TRN2", target_bir_lowering=False, debug=False, num_devices=8)

    xt_d = nc.dram_tensor("xt", [D, T], BF16, kind="ExternalInput")
    wqp_d = nc.dram_tensor("wqp", [D * NH * HD], BF16, kind="ExternalInput")
    wkp_d = nc.dram_tensor("wkp", [D * KVW], BF16, kind="ExternalInput")
    wvp_d = nc.dram_tensor("wvp", [D * KVW], BF16, kind="ExternalInput")
    wop_d = nc.dram_tensor("wop", [NH * HD * D], BF16, kind="ExternalInput")
    cost_d = nc.dram_tensor("cost", [HD, T], F32, kind="ExternalInput")
    sint_d = nc.dram_tensor("sint", [HD, T], F32, kind="ExternalInput")
    maskt_d = nc.dram_tensor("maskt", [S, T], BF16, kind="ExternalInput")
    swap_d = nc.dram_tensor("swapm", [P, P], BF16, kind="ExternalInput")
    onesmat_d = nc.dram_tensor("onesmat", [P, P], BF16, kind="ExternalInput")
    out_d = nc.dram_tensor("out", [T, D], F32, kind="ExternalOutput")

    def grp_offsets(groups):
        offs = []
        off = 0
        for grp in groups:
            offs.append(off)
            off += DT * P * len(grp) * P
        return offs

    qoffs = grp_offsets(qgroups)
    koffs = grp_offsets(kgroups)

    wqp = wqp_d.ap()
    wkp = wkp_d.ap()
    wvp = wvp_d.ap()
    wop = wop_d.ap()

    with tile.TileContext(nc) as tc, ExitStack() as ctx:
        persist = ctx.enter_context(tc.tile_pool(name="persist", bufs=1))
        wpool = ctx.enter_context(tc.tile_pool(name="wpool", bufs=3))
        dramp = ctx.enter_context(tc.tile_pool(name="dramp", bufs=1, space="DRAM"))

        # ---- constants ----
        swap_sb = persist.tile([P, P], BF16, name="swap_sb")
        nc.sync.dma_start(swap_sb[:], swap_d.ap()[:])
        cost_sb = persist.tile([HD, T], F32, name="cost_sb")
        nc.sync.dma_start(cost_sb[:], cost_d.ap()[:])
        sint_sb = persist.tile([HD, T], F32, name="sint_sb")
        nc.sync.dma_start(sint_sb[:], sint_d.ap()[:])
        onesmat_sb = persist.tile([P, P], BF16, name="onesmat_sb")
        nc.sync.dma_start(onesmat_sb[:], onesmat_d.ap()[:])

        kvink = dramp.tile([KVW, T], BF16, name="kvink")
        kvoutk = dramp.tile([4 * KVW, T], BF16, name="kvoutk")
        kvinv = dramp.tile([KVW, T], BF16, name="kvinv")
        kvoutv = dramp.tile([4 * KVW, T], BF16, name="kvoutv")
        kvinv_flat = kvinv[:].rearrange("a b -> (a b)")
        kvoutv_flat = kvoutv[:].rearrange("a b -> (a b)")

        qt = [persist.tile([P, T], BF16, name=f"qt_{h}") for h in range(NH)]

        with tc.tile_pool(name="xtp", bufs=1) as xtp, \
             tc.tile_pool(name="rot", bufs=2) as rot, \
             tc.tile_pool(name="wproj", bufs=1) as wproj, \
             tc.tile_pool(name="psP", bufs=1, space="PSUM") as psP:

            def rotary(raw_ps, dst_bf16, nm):
                """Interleaved rotary on a [P, T] feature-transposed PSUM tile."""
                raw = rot.tile([P, T], BF16, tag="raw", bufs=6, name=f"raw_{nm}")
                nc.scalar.copy(raw[:], raw_ps[:])
                sw_ps = psP.tile([P, T], F32, tag="swp", bufs=2, name=f"swp_{nm}")
                nc.tensor.matmul(sw_ps[:], swap_sb[:], raw[:], start=True, stop=True)
                t1 = rot.tile([P, T], F32, tag="t1", bufs=4, name=f"t1_{nm}")
                nc.vector.tensor_mul(t1[:], raw[:], cost_sb[:])
                t2 = rot.tile([P, T], F32, tag="t2", bufs=4, name=f"t2_{nm}")
                nc.vector.tensor_mul(t2[:], sw_ps[:], sint_sb[:])
                nc.vector.tensor_add(dst_bf16[:], t1[:], t2[:])

            xt_sb = [xtp.tile([P, T], BF16, name=f"xt_sb_{d}") for d in range(DT)]
            xt_loaded = [False] * DT

            def load_xt(d):
                if not xt_loaded[d]:
                    (nc.sync, nc.scalar, nc.gpsimd)[d % 3].dma_start(
                        xt_sb[d][:], xt_d.ap()[d * P:(d + 1) * P, :])
                    xt_loaded[d] = True

            # ---- K^T projection (local tokens) + rotary ----
            ktloc = [xtp.tile([P, T], BF16, name=f"ktloc_{kvh}")
                     for kvh in range(NKV)]
            for gi, grp in enumerate(kgroups):
                w = len(grp) * P
                kps = [psP.tile([P, T], F32, tag=f"pj{j}", bufs=2,
                                name=f"kps_{gi}_{j}") for j in range(len(grp))]
                for d in range(DT):
                    wrow = wproj.tile([P, 3 * P], BF16, tag="wkv", bufs=12,
                                      name=f"wk_{gi}_{d}")
                    off = koffs[gi] + d * P * w
                    (nc.sync, nc.scalar, nc.gpsimd)[d % 3].dma_start(
                        wrow[:, :w],
                        wkp[off:off + P * w].rearrange("(p f) -> p f", p=P))
                    load_xt(d)
                    for j in range(len(grp)):
                        nc.tensor.matmul(
                            kps[j][:], wrow[:, j * HD:(j + 1) * HD], xt_sb[d][:],
                            start=(d == 0), stop=(d == DT - 1))
                for j, kvh in enumerate(grp):
                    rotary(kps[j], ktloc[kvh], f"k{kvh}")

            # ---- K^T pack + AllGather (overlaps the V projection) ----
            for kvh in range(NKV):
                nc.sync.dma_start(kvink[kvh * HD:(kvh + 1) * HD, :], ktloc[kvh][:])
            nc.gpsimd.collective_compute(
                "AllGather",
                mybir.AluOpType.bypass,
                replica_groups=[[0, 1, 2, 3], [4, 5, 6, 7]],
                ins=[kvink[:].opt()],
                outs=[kvoutk[:].opt()],
            )

            # ---- V projection (local tokens), [token, feature] layout ----
            vtloc = [xtp.tile([P, KVW], BF16, name=f"vtloc_{ts}")
                     for ts in range(TS)]
            for gi, (eh, tss) in enumerate(vgroups):
                vps = [psP.tile([P, 512], F32, tag=f"pj{j}", bufs=2,
                                name=f"vps_{gi}_{j}") for j in range(len(tss))]
                for d in range(DT):
                    wrow = wproj.tile([P, 512], BF16, tag="wvr", bufs=12,
                                      name=f"wv_{gi}_{d}")
                    off = (eh * DT + d) * P * 512
                    (nc.sync, nc.scalar, nc.gpsimd)[d % 3].dma_start(
                        wrow[:],
                        wvp[off:off + P * 512].rearrange("(p f) -> p f", p=P))
                    for j, ts in enumerate(tss):
                        nc.tensor.matmul(
                            vps[j][:], xt_sb[d][:, ts * P:(ts + 1) * P], wrow[:],
                            start=(d == 0), stop=(d == DT - 1))
                for j, ts in enumerate(tss):
                    nc.scalar.copy(vtloc[ts][:, eh * 512:(eh + 1) * 512], vps[j][:])

            # ---- V pack + AllGather (overlaps the Q projection) ----
            # V is stored as [kvh][ts] blocks of [128 tokens, 128 features]
            # so the gathered per-(kvh,chunk) slices are contiguous.
            for kvh in range(NKV):
                for ts in range(TS):
                    off = (kvh * TS + ts) * P * HD
                    nc.sync.dma_start(
                        kvinv_flat[off:off + P * HD]
                        .rearrange("(p f) -> p f", p=P),
                        vtloc[ts][:, kvh * HD:(kvh + 1) * HD])

            nc.gpsimd.collective_compute(
                "AllGather",
                mybir.AluOpType.bypass,
                replica_groups=[[0, 1, 2, 3], [4, 5, 6, 7]],
                ins=[kvinv[:].opt()],
                outs=[kvoutv[:].opt()],
            )

            # ---- Q^T projection + rotary (overlaps the AllGather) ----
            for gi, grp in enumerate(qgroups):
                w = len(grp) * P
                qps = [psP.tile([P, T], F32, tag=f"pj{j}", bufs=2,
                                name=f"qps_{gi}_{j}") for j in range(len(grp))]
                for d in range(DT):
                    wrow = wproj.tile([P, 3 * P], BF16, tag="wq", bufs=24,
                                      name=f"wq_{gi}_{d}")
                    off = qoffs[gi] + d * P * w
                    (nc.sync, nc.scalar, nc.gpsimd)[d % 3].dma_start(
                        wrow[:, :w],
                        wqp[off:off + P * w].rearrange("(p f) -> p f", p=P))
                    for j in range(len(grp)):
                        nc.tensor.matmul(
                            qps[j][:], wrow[:, j * HD:(j + 1) * HD], xt_sb[d][:],
                            start=(d == 0), stop=(d == DT - 1))
                for j, h in enumerate(grp):
                    rotary(qps[j], qt[h], f"q{h}")

        tc.no_sync_barrier()

        # Diagonal-schedule mask blocks: key chunk i is masked (per-core
        # data) only against local query position i//4.
        maskt_sb = []
        for i in range(NCH):
            t = persist.tile([P, P], BF16, name=f"maskt_sb_{i}")
            p0 = i // 4
            nc.sync.dma_start(
                t[:], maskt_d.ap()[i * P:(i + 1) * P, p0 * P:(p0 + 1) * P])
            maskt_sb.append(t)

        # ---- attention, streaming gathered K^T / V per kv head ----
        # Key chunk i covers local query positions i//4 .. 3 (a contiguous
        # suffix of the position-major qt tile): one score matmul of width
        # T - 128*(i//4), mask add on its first 128-col block only.
        att = [persist.tile([P, T], BF16, name=f"att_{h}") for h in range(NH)]
        with tc.tile_pool(name="kvp", bufs=1) as kvp, \
             tc.tile_pool(name="atw", bufs=1) as work, \
             tc.tile_pool(name="psA", bufs=1, space="PSUM") as psA:

            def load_kv(kvh):
                ktl = {}
                vtl = {}
                for r in range(4):
                    kt_t = kvp.tile([P, T], BF16, tag="kt", bufs=8,
                                    name=f"kt_{kvh}_{r}")
                    nc.sync.dma_start(
                        kt_t[:],
                        kvoutk[r * KVW + kvh * HD: r * KVW + (kvh + 1) * HD, :])
                    ktl[r] = kt_t
                    for ts in range(TS):
                        i = 4 * ts + r   # global chunk owned by core r, pos ts
                        vt_t = kvp.tile([P, HD], BF16, tag="vts", bufs=2 * NCH,
                                        name=f"vt_{kvh}_{r}_{ts}")
                        off = r * KVW * T + (kvh * TS + ts) * P * HD
                        nc.sync.dma_start(
                            vt_t[:],
                            kvoutv_flat[off:off + P * HD]
                            .rearrange("(p f) -> p f", p=P))
                        vtl[i] = vt_t
                return ktl, vtl

            def emit_scores(hs, ktl):
                """Scores + strided exp + GPSIMD diagonal triangle multiply.
                Strips of a chunk pair sit at fixed T-column slots of a
                2-bank PSUM tile (a matmul may not cross a bank boundary)."""
                et = {0: {}, 1: {}}   # et[j][g, half] -> exp tile
                for g in range(4):
                    w = T - g * P
                    for half in range(2):
                        for j, h in enumerate(hs):
                            s2 = psA.tile([P, 2 * T], F32, tag="s", bufs=2,
                                          name=f"s_{h}_{g}_{half}")
                            for m in range(2):
                                i = 4 * g + 2 * half + m
                                nc.tensor.matmul(
                                    s2[:, m * T:m * T + w],
                                    ktl[i % 4][:, (i // 4) * P:(i // 4 + 1) * P],
                                    qt[h][:, g * P:],
                                    start=True, stop=True)
                            e2 = work.tile([P, 2 * T], BF16, tag="et",
                                           bufs=32, name=f"e_{h}_{g}_{half}")
                            nc.scalar.activation(
                                e2[:].rearrange("p (m c) -> p m c",
                                                m=2)[:, :, :w],
                                s2[:].rearrange("p (m c) -> p m c",
                                                m=2)[:, :, :w],
                                mybir.ActivationFunctionType.Exp,
                                scale=SCALE)
                            # causal cut: multiply the diagonal 128-col
                            # block by a {0,1} triangle (exact in bf16);
                            # runs on the otherwise-idle GPSIMD engine.
                            for m in range(2):
                                i = 4 * g + 2 * half + m
                                eng = nc.gpsimd if i % 2 == 0 else nc.vector
                                eng.tensor_mul(
                                    e2[:, m * T:m * T + P],
                                    e2[:, m * T:m * T + P],
                                    maskt_sb[i][:])
                            et[j][(g, half)] = e2
                return et

            def emit_av(hs, av_ps, zb_ps, et, vtl):
                """A*V and Z accumulation + normalization for one head pair."""
                for j, h in enumerate(hs):
                    for stat_ones in (False, True):
                        dst = zb_ps[j] if stat_ones else av_ps[j]
                        for i in range(NCH):
                            g, rem = i // 4, i % 4
                            e2 = et[j][(g, rem // 2)]
                            m = rem % 2
                            stat = onesmat_sb[:] if stat_ones else vtl[i][:]
                            # One suffix-wide matmul per chunk: it only
                            # touches positions >= g, so per-position
                            # accumulation falls out of the width. start
                            # fires once (chunk 0 spans the full bank),
                            # stop once on the final chunk.
                            nc.tensor.matmul(
                                dst[:, g * P:],
                                stat,
                                e2[:, m * T:m * T + (T - g * P)],
                                start=(i == 0), stop=(i == NCH - 1))
                    rzb = work.tile([P, T], F32, tag="rzbs", bufs=2,
                                    name=f"rzbs_{h}")
                    nc.vector.reciprocal_approx_fast(out=rzb[:],
                                                     in_=zb_ps[j][:])
                    nc.vector.tensor_mul(att[h][:], av_ps[j][:], rzb[:])

            # Software pipeline across (kvh, sub): AV/Z of the previous head
            # pair is emitted after the scores of the current one, so the ACT
            # exp of pair k overlaps the PE AV/Z of pair k-1 instead of
            # stalling the PE (which also kept re-tripping the HAM throttle).
            pending = None
            for kvh in range(NKV):
                ktl, vtl = load_kv(kvh)
                for sub in range(cfg.NREP // 2):
                    hs = [kvh * cfg.NREP + sub * 2, kvh * cfg.NREP + sub * 2 + 1]
                    av_ps = {}
                    zb_ps = {}
                    for j, h in enumerate(hs):
                        av_ps[j] = psA.tile([P, T], F32, tag="av", bufs=2,
                                            name=f"av_{h}")
                        zb_ps[j] = psA.tile([P, T], F32, tag="zb", bufs=2,
                                            name=f"zb_{h}")
                    et = emit_scores(hs, ktl)
                    if pending is not None:
                        emit_av(*pending)
                    pending = (hs, av_ps, zb_ps, et, vtl)
            emit_av(*pending)

        tc.no_sync_barrier()

        # ---- output projection ----
        with tc.tile_pool(name="osbp", bufs=1) as osbp, \
             tc.tile_pool(name="psW", bufs=1, space="PSUM") as psW:
            for douth in range(NDO):
                ops = [psW.tile([P, 512], F32, tag=f"pw{tt}", bufs=2,
                                name=f"ops_{douth}_{tt}") for tt in range(TS)]
                for e in range(NH):
                    wrow = wpool.tile([P, 512], BF16, tag="wo", bufs=12,
                                      name=f"wo_{douth}_{e}")
                    off = (douth * NH + e) * P * 512
                    (nc.sync, nc.scalar, nc.gpsimd)[e % 3].dma_start(
                        wrow[:],
                        wop[off:off + P * 512].rearrange("(p f) -> p f", p=P))
                    for tt in range(TS):
                        nc.tensor.matmul(
                            ops[tt][:], att[e][:, tt * P:(tt + 1) * P], wrow[:],
                            start=(e == 0), stop=(e == NH - 1))
                for tt in range(TS):
                    osb = osbp.tile([P, 512], F32, tag="osb", bufs=4,
                                    name=f"osb_{douth}_{tt}")
                    nc.scalar.copy(osb[:], ops[tt][:])
                    nc.sync.dma_start(
                        out_d.ap()[tt * P:(tt + 1) * P, douth * 512:(douth + 1) * 512],
                        osb[:])

    nc.compile()
    return nc


def owned_tokens(j, cfg: Cfg):
    """Strided query chunks {j, 4+j, 8+j, 12+j}, position-major."""
    return np.concatenate([
        np.arange(P) + P * (4 * p + j) for p in range(cfg.TS)])


def make_in_maps(x, freqs_cis, mask, wq, wk, wv, wo, cfg: Cfg):
    S, D, T, HD, DT = cfg.S, cfg.D, cfg.T, cfg.HD, cfg.DT
    NEH = cfg.NKV * HD // 512
    NDO = D // 512
    SCALE = np.float32(1.0) / np.float32(np.sqrt(np.float32(HD)))
    x = np.asarray(x, np.float32)
    fc = np.asarray(freqs_cis, np.float32)
    mask = np.asarray(mask, np.float32)
    wqt = np.asarray(wq, np.float32).T.astype(NPBF16)   # [D, NH*HD]
    wkt = np.asarray(wk, np.float32).T.astype(NPBF16)   # [D, KVW]
    wvt = np.asarray(wv, np.float32).T.astype(NPBF16)
    wot = np.asarray(wo, np.float32).T.astype(NPBF16)   # [NH*HD, D]

    wqp = pack_colgroups(wqt, groups_of3(cfg.NH), DT)
    wkp = pack_colgroups(wkt, groups_of3(cfg.NKV), DT)
    wvp = np.concatenate([
        np.ascontiguousarray(wvt[d * P:(d + 1) * P, eh * 512:(eh + 1) * 512])
        .reshape(-1)
        for eh in range(NEH) for d in range(DT)])
    wop = np.concatenate([
        np.ascontiguousarray(wot[e * P:(e + 1) * P, douth * 512:(douth + 1) * 512])
        .reshape(-1)
        for douth in range(NDO) for e in range(cfg.NH)])

    swapm = np.zeros((P, P), np.float32)
    for i in range(P // 2):
        swapm[2 * i, 2 * i + 1] = 1.0
        swapm[2 * i + 1, 2 * i] = 1.0
    swapm = swapm.astype(NPBF16)
    onesmat = np.ones((P, P), NPBF16)

    in_maps = []
    for c in range(8):
        b, j = c // 4, c % 4
        sl = owned_tokens(j, cfg)
        xt = np.ascontiguousarray(x[b, sl, :].T).astype(NPBF16)
        cost = np.repeat(fc[sl, :, 0].T, 2, axis=0).astype(np.float32)
        sint = np.repeat(fc[sl, :, 1].T, 2, axis=0).astype(np.float32)
        sint[0::2, :] *= -1.0
        # {0,1} visibility triangle (multiplied into exp(scores), exact in
        # bf16); only the 16 diagonal-schedule blocks are read on device
        maskt = np.ascontiguousarray((mask[sl, :] == 0.0).T.astype(np.float32)
                                     ).astype(NPBF16)
        in_maps.append({
            "xt": xt, "wqp": wqp, "wkp": wkp, "wvp": wvp, "wop": wop,
            "cost": np.ascontiguousarray(cost),
            "sint": np.ascontiguousarray(sint),
            "maskt": maskt, "swapm": swapm, "onesmat": onesmat,
        })
    return in_maps


_NC_CACHE = {}


def kernel_run(x, start_pos, freqs_cis, mask, wq, wk, wv, wo,
               cfg: Cfg = FULL, trace=False):
    in_maps = make_in_maps(x, freqs_cis, mask, wq, wk, wv, wo, cfg)
    if cfg not in _NC_CACHE:
        _NC_CACHE[cfg] = build_nc(cfg)
    nc = _NC_CACHE[cfg]
    res = run_bass_kernel_spmd(nc, in_maps, core_ids=list(range(8)), trace=trace)
    full = np.empty((2, cfg.S, cfg.D), np.float32)
    for c in range(8):
        b, j = c // 4, c % 4
        full[b, owned_tokens(j, cfg), :] = res.results[c]["out"]
    return full, res


def kernel(x, start_pos=None, freqs_cis=None, mask=None, wq=None, wk=None,
           wv=None, wo=None):
    full, _ = kernel_run(x, start_pos, freqs_cis, mask, wq, wk, wv, wo)
    return full



# revision 24
# speedup vs baseline: 1.0533x; 1.0427x over previous
"""Trainium2 Bass kernel for a GQA attention layer (B=2, S=2048, D=4096,
32 Q heads / 8 KV heads, rotary, additive causal mask), SPMD across 8
NeuronCores.

Sharding: core c = (batch b=c//4, stripe j=c%4) owns the STRIDED query
chunk set {j, 4+j, 8+j, 12+j} (128 tokens each, position-major order).
This balances causal work exactly across cores while keeping one uniform
SPMD program: at local query position p the schedule always covers key
chunks 0..4p+3; chunks above the core's own diagonal arrive fully masked
in that core's mask data and contribute exp(-inf)=0.

K/V projections are computed for local tokens only and shared within
each batch's 4 cores via one AllGather (global key chunk i lives in
gathered slot r=i%4, sub-chunk i//4). Attention computes transposed
scores (S^T = K^T-chunk.T @ Q^T); for key chunk i only the query suffix
from position i//4 is computed (one matmul of width 512-128*(i//4)), and
only the first 128-column block (the diagonal) gets a mask add on the
DVE. exp(S^T) feeds the A*V matmul as the moving operand with
region-aligned per-position accumulation; the softmax denominator
accumulates on the PE via an all-ones stationary operand and is applied
after A*V (logits are bounded, so no max subtraction). The wo projection
produces each core's 512 output rows, unshuffled on the host.

Weights are host-packed so every [128, w] stationary tile is a single
contiguous DMA.
"""

import os
import sys
from contextlib import ExitStack
from dataclasses import dataclass

import numpy as np

if os.path.isdir("/opt/trn_rl_repo") and "/opt/trn_rl_repo" not in sys.path:
    sys.path.insert(0, "/opt/trn_rl_repo")

import ml_dtypes

import concourse.bass as bass
import concourse.mybir as mybir
import concourse.tile as tile
from concourse import bacc
from concourse.bass_utils import run_bass_kernel_spmd

BF16 = mybir.dt.bfloat16
F32 = mybir.dt.float32
NPBF16 = ml_dtypes.bfloat16
P = 128


@dataclass(frozen=True)
class Cfg:
    S: int = 2048      # full sequence
    D: int = 4096      # model dim
    NH: int = 32       # query heads
    NKV: int = 8       # kv heads
    HD: int = 128      # head dim (must equal P)

    @property
    def T(self):
        return self.S // 4

    @property
    def TS(self):
        return self.T // P

    @property
    def DT(self):
        return self.D // P

    @property
    def NREP(self):
        return self.NH // self.NKV


FULL = Cfg()


def groups_of3(n):
    return [list(range(k, min(k + 3, n))) for k in range(0, n, 3)]


def pack_colgroups(wT, groups, DT):
    """wT: [D, E] contraction-major. Flat layout: [group][d][128, w_g]
    contiguous blocks."""
    blocks = []
    for grp in groups:
        c0, w = grp[0] * P, len(grp) * P
        for d in range(DT):
            blocks.append(
                np.ascontiguousarray(wT[d * P:(d + 1) * P, c0:c0 + w]).reshape(-1))
    return np.concatenate(blocks)


def build_nc(cfg: Cfg):
    S, D, NH, NKV, HD = cfg.S, cfg.D, cfg.NH, cfg.NKV, cfg.HD
    T, TS, DT = cfg.T, cfg.TS, cfg.DT
    KVW = NKV * HD
    NCH = 4 * TS
    NEH = KVW // 512               # V feature halves
    NDO = D // 512                 # wo output column groups
    SCALE = float(np.float32(1.0) / np.float32(np.sqrt(np.float32(HD))))

    kgroups = groups_of3(NKV)
    qgroups = groups_of3(NH)
    vgroups = [(eh, tss) for eh in range(NEH) for tss in groups_of3(TS)]

    nc = bacc.Bacc("TRN2", target_bir_lowering=False, debug=False, num_devices=8)

    xt_d = nc.dram_tensor("xt", [D, T], BF16, kind="ExternalInput")
    wqp_d = nc.dram_tensor("wqp", [D * NH * HD], BF16, kind="ExternalInput")
    wkp_d = nc.dram_tensor("wkp", [D * KVW], BF16, kind="ExternalInput")
    wvp_d = nc.dram_tensor("wvp", [D * KVW], BF16, kind="ExternalInput")
    wop_d = nc.dram_tensor("wop", [NH * HD * D], BF16, kind="ExternalInput")
    cost_d = nc.dram_tensor("cost", [HD, T], F32, kind="ExternalInput")
    sint_d = nc.dram_tensor("sint", [HD, T], F32, kind="ExternalInput")
    maskt_d = nc.dram_tensor("maskt", [S, T], BF16, kind="ExternalInput")
    swap_d = nc.dram_tensor("swapm", [P, P], BF16, kind="ExternalInput")
    onesmat_d = nc.dram_tensor("onesmat", [P, P], BF16, kind="ExternalInput")
    out_d = nc.dram_tensor("out", [T, D], F32, kind="ExternalOutput")

    def grp_offsets(groups):
        offs = []
        off = 0
        for grp in groups:
            offs.append(off)
            off += DT * P * len(grp) * P
        return offs

    qoffs = grp_offsets(qgroups)
    koffs = grp_offsets(kgroups)

    wqp = wqp_d.ap()
    wkp = wkp_d.ap()
    wvp = wvp_d.ap()
    wop = wop_d.ap()

    with tile.TileContext(nc) as tc, ExitStack() as ctx:
        persist = ctx.enter_context(tc.tile_pool(name="persist", bufs=1))
        wpool = ctx.enter_context(tc.tile_pool(name="wpool", bufs=3))
        dramp = ctx.enter_context(tc.tile_pool(name="dramp", bufs=1, space="DRAM"))

        # ---- constants ----
        swap_sb = persist.tile([P, P], BF16, name="swap_sb")
        nc.sync.dma_start(swap_sb[:], swap_d.ap()[:])
        cost_sb = persist.tile([HD, T], F32, name="cost_sb")
        nc.sync.dma_start(cost_sb[:], cost_d.ap()[:])
        sint_sb = persist.tile([HD, T], F32, name="sint_sb")
        nc.sync.dma_start(sint_sb[:], sint_d.ap()[:])
        onesmat_sb = persist.tile([P, P], BF16, name="onesmat_sb")
        nc.sync.dma_start(onesmat_sb[:], onesmat_d.ap()[:])

        kvink = dramp.tile([KVW, T], BF16, name="kvink")
        kvoutk = dramp.tile([4 * KVW, T], BF16, name="kvoutk")
        kvinv = dramp.tile([KVW, T], BF16, name="kvinv")
        kvoutv = dramp.tile([4 * KVW, T], BF16, name="kvoutv")
        kvinv_flat = kvinv[:].rearrange("a b -> (a b)")
        kvoutv_flat = kvoutv[:].rearrange("a b -> (a b)")

        qt = [persist.tile([P, T], BF16, name=f"qt_{h}") for h in range(NH)]

        with tc.tile_pool(name="xtp", bufs=1) as xtp, \
             tc.tile_pool(name="rot", bufs=2) as rot, \
             tc.tile_pool(name="wproj", bufs=1) as wproj:

            def rotary(pspool, raw_ps, dst_bf16, nm):
                """Interleaved rotary on a [P, T] feature-transposed PSUM tile."""
                raw = rot.tile([P, T], BF16, tag="raw", bufs=6, name=f"raw_{nm}")
                nc.scalar.copy(raw[:], raw_ps[:])
                sw_ps = pspool.tile([P, T], F32, tag="swp", bufs=2, name=f"swp_{nm}")
                nc.tensor.matmul(sw_ps[:], swap_sb[:], raw[:], start=True, stop=True)
                t1 = rot.tile([P, T], F32, tag="t1", bufs=4, name=f"t1_{nm}")
                nc.vector.tensor_mul(t1[:], raw[:], cost_sb[:])
                t2 = rot.tile([P, T], F32, tag="t2", bufs=4, name=f"t2_{nm}")
                nc.vector.tensor_mul(t2[:], sw_ps[:], sint_sb[:])
                nc.vector.tensor_add(dst_bf16[:], t1[:], t2[:])

            xt_sb = [xtp.tile([P, T], BF16, name=f"xt_sb_{d}") for d in range(DT)]
            xt_loaded = [False] * DT

            def load_xt(d):
                if not xt_loaded[d]:
                    (nc.sync, nc.scalar, nc.gpsimd)[d % 3].dma_start(
                        xt_sb[d][:], xt_d.ap()[d * P:(d + 1) * P, :])
                    xt_loaded[d] = True

            # ---- K^T projection (local tokens) + rotary ----
            ktloc = [xtp.tile([P, T], BF16, name=f"ktloc_{kvh}")
                     for kvh in range(NKV)]
            with tc.tile_pool(name="psK", bufs=1, space="PSUM") as psK:
                for gi, grp in enumerate(kgroups):
                    w = len(grp) * P
                    kps = [psK.tile([P, T], F32, tag=f"pj{j}", bufs=2,
                                    name=f"kps_{gi}_{j}") for j in range(len(grp))]
                    for d in range(DT):
                        wrow = wproj.tile([P, 3 * P], BF16, tag="wkv", bufs=12,
                                          name=f"wk_{gi}_{d}")
                        off = koffs[gi] + d * P * w
                        (nc.sync, nc.scalar, nc.gpsimd)[d % 3].dma_start(
                            wrow[:, :w],
                            wkp[off:off + P * w].rearrange("(p f) -> p f", p=P))
                        load_xt(d)
                        for j in range(len(grp)):
                            nc.tensor.matmul(
                                kps[j][:], wrow[:, j * HD:(j + 1) * HD], xt_sb[d][:],
                                start=(d == 0), stop=(d == DT - 1))
                    for j, kvh in enumerate(grp):
                        rotary(psK, kps[j], ktloc[kvh], f"k{kvh}")

            # ---- K^T pack + AllGather (overlaps the V projection) ----
            for kvh in range(NKV):
                nc.sync.dma_start(kvink[kvh * HD:(kvh + 1) * HD, :], ktloc[kvh][:])
            nc.gpsimd.collective_compute(
                "AllGather",
                mybir.AluOpType.bypass,
                replica_groups=[[0, 1, 2, 3], [4, 5, 6, 7]],
                ins=[kvink[:].opt()],
                outs=[kvoutk[:].opt()],
            )

            # ---- V projection (local tokens), [token, feature] layout ----
            # Single weight pass: all four token sub-chunks accumulate
            # concurrently (own PSUM scope -> 8 banks available).
            vtloc = [xtp.tile([P, KVW], BF16, name=f"vtloc_{ts}")
                     for ts in range(TS)]
            with tc.tile_pool(name="psV", bufs=1, space="PSUM") as psV:
                for eh in range(NEH):
                    vps = [psV.tile([P, 512], F32, tag=f"vj{ts}", bufs=2,
                                    name=f"vps_{eh}_{ts}") for ts in range(TS)]
                    for d in range(DT):
                        wrow = wproj.tile([P, 512], BF16, tag="wvr", bufs=12,
                                          name=f"wv_{eh}_{d}")
                        off = (eh * DT + d) * P * 512
                        (nc.sync, nc.scalar, nc.gpsimd)[d % 3].dma_start(
                            wrow[:],
                            wvp[off:off + P * 512].rearrange("(p f) -> p f", p=P))
                        for ts in range(TS):
                            nc.tensor.matmul(
                                vps[ts][:], xt_sb[d][:, ts * P:(ts + 1) * P],
                                wrow[:],
                                start=(d == 0), stop=(d == DT - 1))
                    for ts in range(TS):
                        nc.scalar.copy(vtloc[ts][:, eh * 512:(eh + 1) * 512],
                                       vps[ts][:])

            # ---- V pack + AllGather (overlaps the Q projection) ----
            # V is stored as [kvh][ts] blocks of [128 tokens, 128 features]
            # so the gathered per-(kvh,chunk) slices are contiguous.
            for kvh in range(NKV):
                for ts in range(TS):
                    off = (kvh * TS + ts) * P * HD
                    nc.sync.dma_start(
                        kvinv_flat[off:off + P * HD]
                        .rearrange("(p f) -> p f", p=P),
                        vtloc[ts][:, kvh * HD:(kvh + 1) * HD])

            nc.gpsimd.collective_compute(
                "AllGather",
                mybir.AluOpType.bypass,
                replica_groups=[[0, 1, 2, 3], [4, 5, 6, 7]],
                ins=[kvinv[:].opt()],
                outs=[kvoutv[:].opt()],
            )

            # ---- Q^T projection + rotary (overlaps the AllGather) ----
            with tc.tile_pool(name="psQ", bufs=1, space="PSUM") as psQ:
                for gi, grp in enumerate(qgroups):
                    w = len(grp) * P
                    qps = [psQ.tile([P, T], F32, tag=f"pj{j}", bufs=2,
                                    name=f"qps_{gi}_{j}") for j in range(len(grp))]
                    for d in range(DT):
                        wrow = wproj.tile([P, 3 * P], BF16, tag="wq", bufs=24,
                                          name=f"wq_{gi}_{d}")
                        off = qoffs[gi] + d * P * w
                        (nc.sync, nc.scalar, nc.gpsimd)[d % 3].dma_start(
                            wrow[:, :w],
                            wqp[off:off + P * w].rearrange("(p f) -> p f", p=P))
                        for j in range(len(grp)):
                            nc.tensor.matmul(
                                qps[j][:], wrow[:, j * HD:(j + 1) * HD],
                                xt_sb[d][:],
                                start=(d == 0), stop=(d == DT - 1))
                    for j, h in enumerate(grp):
                        rotary(psQ, qps[j], qt[h], f"q{h}")

        tc.no_sync_barrier()

        # Diagonal-schedule mask blocks: key chunk i is masked (per-core
        # data) only against local query position i//4.
        maskt_sb = []
        for i in range(NCH):
            t = persist.tile([P, P], BF16, name=f"maskt_sb_{i}")
            p0 = i // 4
            nc.sync.dma_start(
                t[:], maskt_d.ap()[i * P:(i + 1) * P, p0 * P:(p0 + 1) * P])
            maskt_sb.append(t)

        # ---- attention, streaming gathered K^T / V per kv head ----
        # Key chunk i covers local query positions i//4 .. 3 (a contiguous
        # suffix of the position-major qt tile): one score matmul of width
        # T - 128*(i//4), mask add on its first 128-col block only.
        att = [persist.tile([P, T], BF16, name=f"att_{h}") for h in range(NH)]
        with tc.tile_pool(name="kvp", bufs=1) as kvp, \
             tc.tile_pool(name="atw", bufs=1) as work, \
             tc.tile_pool(name="psA", bufs=1, space="PSUM") as psA:

            def load_kv(kvh):
                ktl = {}
                vtl = {}
                for r in range(4):
                    kt_t = kvp.tile([P, T], BF16, tag="kt", bufs=8,
                                    name=f"kt_{kvh}_{r}")
                    nc.sync.dma_start(
                        kt_t[:],
                        kvoutk[r * KVW + kvh * HD: r * KVW + (kvh + 1) * HD, :])
                    ktl[r] = kt_t
                    for ts in range(TS):
                        i = 4 * ts + r   # global chunk owned by core r, pos ts
                        vt_t = kvp.tile([P, HD], BF16, tag="vts", bufs=2 * NCH,
                                        name=f"vt_{kvh}_{r}_{ts}")
                        off = r * KVW * T + (kvh * TS + ts) * P * HD
                        nc.sync.dma_start(
                            vt_t[:],
                            kvoutv_flat[off:off + P * HD]
                            .rearrange("(p f) -> p f", p=P))
                        vtl[i] = vt_t
                return ktl, vtl

            def emit_scores(hs, ktl):
                """Scores + strided exp + GPSIMD diagonal triangle multiply.
                Strips of a chunk pair sit at fixed T-column slots of a
                2-bank PSUM tile (a matmul may not cross a bank boundary)."""
                et = {0: {}, 1: {}}   # et[j][g, half] -> exp tile
                for g in range(4):
                    w = T - g * P
                    for half in range(2):
                        for j, h in enumerate(hs):
                            s2 = psA.tile([P, 2 * T], F32, tag="s", bufs=2,
                                          name=f"s_{h}_{g}_{half}")
                            for m in range(2):
                                i = 4 * g + 2 * half + m
                                nc.tensor.matmul(
                                    s2[:, m * T:m * T + w],
                                    ktl[i % 4][:, (i // 4) * P:(i // 4 + 1) * P],
                                    qt[h][:, g * P:],
                                    start=True, stop=True)
                            e2 = work.tile([P, 2 * T], BF16, tag="et",
                                           bufs=32, name=f"e_{h}_{g}_{half}")
                            nc.scalar.activation(
                                e2[:].rearrange("p (m c) -> p m c",
                                                m=2)[:, :, :w],
                                s2[:].rearrange("p (m c) -> p m c",
                                                m=2)[:, :, :w],
                                mybir.ActivationFunctionType.Exp,
                                scale=SCALE)
                            # causal cut: multiply the diagonal 128-col
                            # block by a {0,1} triangle (exact in bf16);
                            # runs on the otherwise-idle GPSIMD engine.
                            for m in range(2):
                                i = 4 * g + 2 * half + m
                                eng = nc.gpsimd if i % 2 == 0 else nc.vector
                                eng.tensor_mul(
                                    e2[:, m * T:m * T + P],
                                    e2[:, m * T:m * T + P],
                                    maskt_sb[i][:])
                            et[j][(g, half)] = e2
                return et

            def emit_av(hs, av_ps, zb_ps, et, vtl):
                """A*V and Z accumulation + normalization for one head pair."""
                for j, h in enumerate(hs):
                    for stat_ones in (False, True):
                        dst = zb_ps[j] if stat_ones else av_ps[j]
                        for i in range(NCH):
                            g, rem = i // 4, i % 4
                            e2 = et[j][(g, rem // 2)]
                            m = rem % 2
                            stat = onesmat_sb[:] if stat_ones else vtl[i][:]
                            # One suffix-wide matmul per chunk: it only
                            # touches positions >= g, so per-position
                            # accumulation falls out of the width. start
                            # fires once (chunk 0 spans the full bank),
                            # stop once on the final chunk.
                            nc.tensor.matmul(
                                dst[:, g * P:],
                                stat,
                                e2[:, m * T:m * T + (T - g * P)],
                                start=(i == 0), stop=(i == NCH - 1))
                    rzb = work.tile([P, T], F32, tag="rzbs", bufs=2,
                                    name=f"rzbs_{h}")
                    nc.vector.reciprocal_approx_fast(out=rzb[:],
                                                     in_=zb_ps[j][:])
                    nc.vector.tensor_mul(att[h][:], av_ps[j][:], rzb[:])

            # Software pipeline across (kvh, sub): AV/Z of the previous head
            # pair is emitted after the scores of the current one, so the ACT
            # exp of pair k overlaps the PE AV/Z of pair k-1 instead of
            # stalling the PE (which also kept re-tripping the HAM throttle).
            pending = None
            for kvh in range(NKV):
                ktl, vtl = load_kv(kvh)
                for sub in range(cfg.NREP // 2):
                    hs = [kvh * cfg.NREP + sub * 2, kvh * cfg.NREP + sub * 2 + 1]
                    av_ps = {}
                    zb_ps = {}
                    for j, h in enumerate(hs):
                        av_ps[j] = psA.tile([P, T], F32, tag="av", bufs=2,
                                            name=f"av_{h}")
                        zb_ps[j] = psA.tile([P, T], F32, tag="zb", bufs=2,
                                            name=f"zb_{h}")
                    et = emit_scores(hs, ktl)
                    if pending is not None:
                        emit_av(*pending)
                    pending = (hs, av_ps, zb_ps, et, vtl)
            emit_av(*pending)

        tc.no_sync_barrier()

        # ---- output projection ----
        with tc.tile_pool(name="osbp", bufs=1) as osbp, \
             tc.tile_pool(name="psW", bufs=1, space="PSUM") as psW:
            for douth in range(NDO):
                ops = [psW.tile([P, 512], F32, tag=f"pw{tt}", bufs=2,
                                name=f"ops_{douth}_{tt}") for tt in range(TS)]
                for e in range(NH):
                    wrow = wpool.tile([P, 512], BF16, tag="wo", bufs=12,
                                      name=f"wo_{douth}_{e}")
                    off = (douth * NH + e) * P * 512
                    (nc.sync, nc.scalar, nc.gpsimd)[e % 3].dma_start(
                        wrow[:],
                        wop[off:off + P * 512].rearrange("(p f) -> p f", p=P))
                    for tt in range(TS):
                        nc.tensor.matmul(
                            ops[tt][:], att[e][:, tt * P:(tt + 1) * P], wrow[:],
                            start=(e == 0), stop=(e == NH - 1))
                for tt in range(TS):
                    osb = osbp.tile([P, 512], F32, tag="osb", bufs=4,
                                    name=f"osb_{douth}_{tt}")
                    nc.scalar.copy(osb[:], ops[tt][:])
                    nc.sync.dma_start(
                        out_d.ap()[tt * P:(tt + 1) * P, douth * 512:(douth + 1) * 512],
                        osb[:])

    nc.compile()
    return nc


def owned_tokens(j, cfg: Cfg):
    """Strided query chunks {j, 4+j, 8+j, 12+j}, position-major."""
    return np.concatenate([
        np.arange(P) + P * (4 * p + j) for p in range(cfg.TS)])


def make_in_maps(x, freqs_cis, mask, wq, wk, wv, wo, cfg: Cfg):
    S, D, T, HD, DT = cfg.S, cfg.D, cfg.T, cfg.HD, cfg.DT
    NEH = cfg.NKV * HD // 512
    NDO = D // 512
    SCALE = np.float32(1.0) / np.float32(np.sqrt(np.float32(HD)))
    x = np.asarray(x, np.float32)
    fc = np.asarray(freqs_cis, np.float32)
    mask = np.asarray(mask, np.float32)
    wqt = np.asarray(wq, np.float32).T.astype(NPBF16)   # [D, NH*HD]
    wkt = np.asarray(wk, np.float32).T.astype(NPBF16)   # [D, KVW]
    wvt = np.asarray(wv, np.float32).T.astype(NPBF16)
    wot = np.asarray(wo, np.float32).T.astype(NPBF16)   # [NH*HD, D]

    wqp = pack_colgroups(wqt, groups_of3(cfg.NH), DT)
    wkp = pack_colgroups(wkt, groups_of3(cfg.NKV), DT)
    wvp = np.concatenate([
        np.ascontiguousarray(wvt[d * P:(d + 1) * P, eh * 512:(eh + 1) * 512])
        .reshape(-1)
        for eh in range(NEH) for d in range(DT)])
    wop = np.concatenate([
        np.ascontiguousarray(wot[e * P:(e + 1) * P, douth * 512:(douth + 1) * 512])
        .reshape(-1)
        for douth in range(NDO) for e in range(cfg.NH)])

    swapm = np.zeros((P, P), np.float32)
    for i in range(P // 2):
        swapm[2 * i, 2 * i + 1] = 1.0
        swapm[2 * i + 1, 2 * i] = 1.0
    swapm = swapm.astype(NPBF16)
    onesmat = np.ones((P, P), NPBF16)

    in_maps = []
    for c in range(8):
        b, j = c // 4, c % 4
        sl = owned_tokens(j, cfg)
        xt = np.ascontiguousarray(x[b, sl, :].T).astype(NPBF16)
        cost = np.repeat(fc[sl, :, 0].T, 2, axis=0).astype(np.float32)
        sint = np.repeat(fc[sl, :, 1].T, 2, axis=0).astype(np.float32)
        sint[0::2, :] *= -1.0
        # {0,1} visibility triangle (multiplied into exp(scores), exact in
        # bf16); only the 16 diagonal-schedule blocks are read on device
        maskt = np.ascontiguousarray((mask[sl, :] == 0.0).T.astype(np.float32)
                                     ).astype(NPBF16)
        in_maps.append({
            "xt": xt, "wqp": wqp, "wkp": wkp, "wvp": wvp, "wop": wop,
            "cost": np.ascontiguousarray(cost),
            "sint": np.ascontiguousarray(sint),
            "maskt": maskt, "swapm": swapm, "onesmat": onesmat,
        })
    return in_maps


_NC_CACHE = {}


def kernel_run(x, start_pos, freqs_cis, mask, wq, wk, wv, wo,
               cfg: Cfg = FULL, trace=False):
    in_maps = make_in_maps(x, freqs_cis, mask, wq, wk, wv, wo, cfg)
    if cfg not in _NC_CACHE:
        _NC_CACHE[cfg] = build_nc(cfg)
    nc = _NC_CACHE[cfg]
    res = run_bass_kernel_spmd(nc, in_maps, core_ids=list(range(8)), trace=trace)
    full = np.empty((2, cfg.S, cfg.D), np.float32)
    for c in range(8):
        b, j = c // 4, c % 4
        full[b, owned_tokens(j, cfg), :] = res.results[c]["out"]
    return full, res


def kernel(x, start_pos=None, freqs_cis=None, mask=None, wq=None, wk=None,
           wv=None, wo=None):
    full, _ = kernel_run(x, start_pos, freqs_cis, mask, wq, wk, wv, wo)
    return full

